# revision 46
# baseline (speedup 1.0000x reference)
"""Trainium2 Bass kernel for nn_FKRM_85839216378385 (vq_codebook).

Supercode reformulation of the attention: after ln1 (d=3), the pre-q
vector lies on the circle sqrt(3)(u cos t + w sin t) in the plane sum=0,
so every codebook score is A_i + R_i cos(t - phi_i) — a function of ONE
angle t per pixel.  The softmax numerator/denominator are weight-only
1-D periodic functions, re-expressed host-side (FFT deconvolution, a
pure weight transform) as a sum of M=128 von-Mises "supercodes"
   T_d(t) ~= sum_j c_jd exp(rtil cos(t - phi_j)),
so the device keeps the baseline structure (scores matmul -> exp ->
[v|1] contraction -> divide) with a 128-entry codebook instead of 8192.
Device features per pixel are (rho cos t, rho sin t) = cs *
rsqrt(|cs|^2 + eps) with cs a 2-channel linear head off mlp_in's hidden
layer (bias folded into the matmul via a ones row).

Sharding: pixels (image rows) over the 8 cores for the attention branch
AND for the PSF image-fusion branch.  PSF runs in w-partition layout
[w=96 | (b, c, h=12+halo)] so per-core ops are 24-108 columns wide; the
row halo (+-3) is materialized host-side.  The only global coupling is
the per-batch min/max of the cosine map S: each core exports per-core
min/max partials plus conv(S*(EE-HH)), conv(EE-HH), conv(HH) for its
rows, and the gather combines them (fuse is affine in S's normalizer).

Engine plan: Act runs gelu/arsqrt/exp/gelu/arsqrt in function-grouped
windows (5 table loads); PE carries all channel-mixing matmuls plus the
PSF w-box as a band matmul; DVE/Pool split the element-wise chains.
"""

import numpy as np

N_CORES = 8
B, C, H, W = 2, 3, 96, 96
D = 3
NE = 8192
NWIN = 7
PAD = NWIN // 2          # 3
HSL = H // N_CORES       # 12 rows per core (per batch)
HT = HSL + 2 * PAD       # 18 halo'd rows
PIX = B * HSL * W        # 2304 pixels per core
NVAR = float(NWIN * NWIN)          # 49
SCALE = float(D) ** -0.5
PCS = [(0, 512), (512, 512), (1024, 512), (1536, 512), (2048, 256)]
M_SC = 128               # supercodes
RTIL = 8.0               # von-Mises kernel concentration
M_CUT = 31               # deconvolution harmonic cutoff
BCT = B * C * HT         # 108
BCO = B * C * HSL        # 72


def _build_program():
    import sys
    if "/opt/trn_rl_repo" not in sys.path:
        sys.path.insert(0, "/opt/trn_rl_repo")
    import concourse.bass as bass
    import concourse.mybir as mybir
    import concourse.tile as tile
    from concourse import bacc
    import concourse.bass_isa as bass_isa
    from contextlib import ExitStack

    f32 = mybir.dt.float32
    f32r = mybir.dt.float32r
    AF = mybir.ActivationFunctionType
    ALU = mybir.AluOpType
    AX = mybir.AxisListType
    u32 = mybir.dt.uint32
    ROP = bass_isa.ReduceOp

    nc = bacc.Bacc("TRN2", target_bir_lowering=False, debug=False,
                   num_devices=N_CORES)

    # ---------------- dram I/O ----------------
    d_xcm = nc.dram_tensor("front_cm", [D + 1, PIX], f32r,
                           kind="ExternalInput")
    d_fp = nc.dram_tensor("fpsf", [W, BCT], f32r, kind="ExternalInput")
    d_bp = nc.dram_tensor("bpsf", [W, BCT], f32r, kind="ExternalInput")
    d_k2 = nc.dram_tensor("k2", [2, M_SC], f32r, kind="ExternalInput")
    d_cj = nc.dram_tensor("cj", [M_SC, 4], f32r, kind="ExternalInput")
    d_band = nc.dram_tensor("band", [W, W], f32r, kind="ExternalInput")
    # small weights packed into one [8, 64] tensor:
    # [0:3,0:6]=w1T [0:7,6:8]=cswT7(row6=csb) [0:2,8:10]=ones22
    # [0:4,10:13]=fuse4T(row3=fuseb) [0:3,13:16]=ones33/3
    # [0:6,16:19]=mo2cT [0:4,19:57]=hdsel38 (19:25=mo1T|0, 51:57=sel436)
    d_wpack = nc.dram_tensor("wpack", [8, 64], f32r, kind="ExternalInput")
    # bias vectors packed into one [6, 6] tensor: cols b1,-,mob1,mob2c,-,eps
    d_vpack = nc.dram_tensor("vpack", [6, 6], f32, kind="ExternalInput")
    # 45 conv weights + 12 conv biases (e,f,g,h x 3 channels)
    d_cw = nc.dram_tensor("cw", [57], f32, kind="ExternalInput")

    d_oa = nc.dram_tensor("out_a", [D, PIX], f32, kind="ExternalOutput")
    # PSF shard outputs: conv(S*(EE-HH)) | conv(EE-HH) | conv(HH)
    d_ps = nc.dram_tensor("psf_out", [W, 3 * B * 3 * HSL], f32,
                          kind="ExternalOutput")
    d_mm = nc.dram_tensor("mm_out", [1, 2 * B], f32, kind="ExternalOutput")

    def bcast_ap(handle, n):
        a = handle[:]
        return bass.AP(tensor=a.tensor, offset=a.offset, ap=[[0, 96], [1, n]])

    with tile.TileContext(nc) as tc, ExitStack() as ctx:
        consts = ctx.enter_context(tc.tile_pool(name="consts", bufs=1))
        data = ctx.enter_context(tc.tile_pool(name="data", bufs=1))
        psf = ctx.enter_context(tc.tile_pool(name="psf", bufs=1))
        psft = ctx.enter_context(tc.tile_pool(name="psft", bufs=2))

        # ---------------- constants to SBUF ----------------
        X_sb = data.tile([D + 1, PIX], f32r, tag="X")
        nc.sync.dma_start(out=X_sb, in_=d_xcm[:, :])
        wp_sb = consts.tile([8, 64], f32r)
        nc.sync.dma_start(out=wp_sb, in_=d_wpack[:, :])
        vp_sb = consts.tile([6, 6], f32)
        nc.sync.dma_start(out=vp_sb, in_=d_vpack[:, :])
        k2_sb = consts.tile([2, M_SC], f32r)
        nc.sync.dma_start(out=k2_sb, in_=d_k2[:, :])
        cj_sb = consts.tile([M_SC, 4], f32r)
        nc.sync.dma_start(out=cj_sb, in_=d_cj[:, :])
        magic_sb = consts.tile([W, BCO], mybir.dt.uint32)
        nc.gpsimd.memset(magic_sb, 0x5F3759DF)
        band_sb = consts.tile([W, W], f32r)
        nc.gpsimd.dma_start(out=band_sb, in_=d_band[:, :])
        cw_sb = consts.tile([96, 57], f32)
        nc.gpsimd.dma_start(out=cw_sb, in_=bcast_ap(d_cw, 57))

        w1T_sb = wp_sb[0:3, 0:6]
        cswT_sb = wp_sb[0:7, 6:8]
        ones22_sb = wp_sb[0:2, 8:10]
        fuse4_sb = wp_sb[0:4, 10:13]
        ones33_sb = wp_sb[0:3, 13:16]
        mo2cT_sb = wp_sb[0:6, 16:19]
        hdsel_sb = wp_sb[0:4, 19:57]
        b1_sb = vp_sb[0:6, 0:1]
        mob1_sb = vp_sb[0:6, 2:3]
        mob2c_sb = vp_sb[0:3, 3:4]
        eps_sb = vp_sb[0:6, 5:6]

        # ---------------- PSF shard staging (w-partition layout) --------
        pf = psf.tile([W, B, C, HT], f32r, tag="pf")
        pb = psf.tile([W, B, C, HT], f32r, tag="pb")
        nc.sync.dma_start(out=pf.rearrange("p b c t -> p (b c t)"),
                          in_=d_fp[:, :])
        nc.sync.dma_start(out=pb.rearrange("p b c t -> p (b c t)"),
                          in_=d_bp[:, :])

        # ---------------- SBUF attention tiles ----------------
        # matmul-moving tensors need base partition == the stationary's
        # (0): one tile each; non-moving intermediates share one tile.
        h_sb = data.tile([7, PIX], f32r, tag="h")
        sq_sb = data.tile([2, PIX], f32r, tag="sqcs")
        CS_sb = data.tile([2, PIX], f32r, tag="CS")
        nd_sb = data.tile([4, PIX], f32r, tag="nd")
        h2_sb = data.tile([6, PIX], f32r, tag="h2")
        y2_sb = data.tile([4, PIX], f32r, tag="y2")
        sq2_sb = data.tile([3, PIX], f32r, tag="sq2")
        ex_sb = data.tile([M_SC, PIX], f32r, tag="ex")
        cs_sb = data.tile([2, PIX], f32r, tag="cs")
        rstd_sb = data.tile([2, PIX], f32r, tag="rstd")
        r6_sb = data.tile([6, PIX], f32r, tag="r6")
        xg_sb = data.tile([6, PIX], f32r, tag="xg")
        r32_sb = data.tile([3, PIX], f32r, tag="r32")
        oa_sb = data.tile([3, PIX], f32r, tag="oa")

        # ---------------- PSF shard tiles (w-layout) ----------------
        spf = psf.tile([W, B, C, HT], f32r, tag="spf")
        spb = psf.tile([W, B, C, HT], f32r, tag="spb")
        hx_f = psf.tile([W, B, C, HT], f32, tag="hx_f")
        hx_b = psf.tile([W, B, C, HT], f32, tag="hx_b")
        hx_f2 = psf.tile([W, B, C, HT], f32, tag="hx_f2")
        hx_b2 = psf.tile([W, B, C, HT], f32, tag="hx_b2")
        t1_ = psf.tile([W, B, C, HT], f32, tag="t1_")
        t2_ = psf.tile([W, B, C, HT], f32, tag="t2_")
        brh_f = psf.tile([W, B, C, HSL], f32, tag="brh_f")
        brh_b = psf.tile([W, B, C, HSL], f32, tag="brh_b")
        brh_f2 = psf.tile([W, B, C, HSL], f32, tag="brh_f2")
        brh_b2 = psf.tile([W, B, C, HSL], f32, tag="brh_b2")
        v_f = psf.tile([W, B, C, HSL], f32, tag="v_f")
        v_b = psf.tile([W, B, C, HSL], f32, tag="v_b")
        r_f = psf.tile([W, B, C, HSL], f32, tag="r_f")
        r_b = psf.tile([W, B, C, HSL], f32, tag="r_b")
        m_f = psf.tile([W, B, C, HSL], f32, tag="m_f")
        m_b = psf.tile([W, B, C, HSL], f32, tag="m_b")
        sd_b = psf.tile([W, B, C, HSL], f32, tag="sd_b")
        xnf = psf.tile([W, B, C, HSL], f32, tag="xnf")
        xnb = psf.tile([W, B, C, HSL], f32, tag="xnb")
        xad = psf.tile([W, B, C, HSL], f32, tag="xad")
        EE = psf.tile([W, B, C, HSL], f32, tag="EE")
        FF = psf.tile([W, B, C, HSL], f32, tag="FF")
        GG = psf.tile([W, B, C, HSL], f32, tag="GG")
        HH = psf.tile([W, B, C, HSL], f32, tag="HH")
        A3 = psf.tile([W, B, C, HSL], f32, tag="A3")
        P3 = psf.tile([W, B, C, HSL], f32, tag="P3")
        psO = psf.tile([W, 3, B, C, HSL], f32, tag="psO")
        dot = psf.tile([W, B, HSL], f32, tag="dot")
        f2 = psf.tile([W, B, HSL], f32, tag="f2")
        g2 = psf.tile([W, B, HSL], f32, tag="g2")
        S = psf.tile([W, B, HSL], f32, tag="S")

        # -------- PSUM pools: rotated small-out pool + score + box ----
        with tc.tile_pool(name="psA", bufs=4, space="PSUM") as psA, \
             tc.tile_pool(name="psS", bufs=2, space="PSUM") as psS, \
             tc.tile_pool(name="psV", bufs=2, space="PSUM") as psV:

            def small(c, p, nm):
                t = psA.tile([12, 512], f32, tag="sm", name=f"{nm}_{c}")
                return t[0:p, 0:PCS[c][1]]

            def csl(c):
                off, n = PCS[c]
                return slice(off, off + n)

            NCH = len(PCS)

            # ================= head =================
            # ones rows for bias folds arrive by DMA (X row 3 is ones)
            nc.sync.dma_start(out=h_sb[6:7, :], in_=d_xcm[3:4, :])
            nc.sync.dma_start(out=y2_sb[3:4, :], in_=d_xcm[3:4, :])
            hps = {}
            for c in range(NCH):
                hps[c] = small(c, 6, "hps")
                nc.tensor.matmul(hps[c], w1T_sb, X_sb[0:3, csl(c)],
                                 start=True, stop=True)
            for c in range(NCH):
                nc.scalar.activation(h_sb[0:6, csl(c)], hps[c],
                                     AF.Gelu_apprx_tanh, bias=b1_sb)
            # PSF squares (w-layout, 108 cols)
            pff = pf.rearrange("p b c t -> p (b c t)")
            pbf = pb.rearrange("p b c t -> p (b c t)")
            nc.vector.tensor_mul(spf.rearrange("p b c t -> p (b c t)"),
                                 pff, pff)
            nc.gpsimd.tensor_mul(spb.rearrange("p b c t -> p (b c t)"),
                                 pbf, pbf)

            csp = {}
            for c in range(NCH):
                csp[c] = small(c, 2, "csp")
                nc.tensor.matmul(csp[c], cswT_sb, h_sb[:, csl(c)],
                                 start=True, stop=True)
            for c in range(NCH):
                nc.vector.tensor_copy(cs_sb[:, csl(c)], csp[c])
            for c in range(NCH):
                nc.gpsimd.tensor_mul(sq_sb[:, csl(c)], cs_sb[:, csl(c)],
                                     cs_sb[:, csl(c)])
            varp = {}
            for c in range(NCH):
                varp[c] = small(c, 2, "varp")
                nc.tensor.matmul(varp[c], ones22_sb, sq_sb[:, csl(c)],
                                 start=True, stop=True)

            # ---- PSF hbox: band matmul over the w partitions ----
            def hboxw(dst, src):
                sfl = src.rearrange("p b c t -> p (b c t)")
                dfl = dst.rearrange("p b c t -> p (b c t)")
                bank = psV.tile([W, BCT], f32, tag="vb",
                                name=f"hb_{id(dst)}")
                nc.tensor.matmul(bank, band_sb, sfl, start=True, stop=True)
                nc.vector.tensor_copy(dfl, bank)

            hboxw(hx_f, pf)
            hboxw(hx_b, pb)
            hboxw(hx_f2, spf)
            hboxw(hx_b2, spb)

            # ---- PSF vbox: 4-op tree over the h (free) axis ----
            def vboxh(dst, x, eng):
                eng.tensor_add(t1_[:, :, :, 0:HT - 1], x[:, :, :, 0:HT - 1],
                               x[:, :, :, 1:HT])
                eng.tensor_add(t2_[:, :, :, 0:HT - 3], t1_[:, :, :, 0:HT - 3],
                               t1_[:, :, :, 2:HT - 1])
                eng.tensor_add(dst, t2_[:, :, :, 0:HSL],
                               t1_[:, :, :, 4:4 + HSL])
                eng.tensor_add(dst, dst, x[:, :, :, 6:6 + HSL])

            vboxh(brh_f, hx_f, nc.vector)
            vboxh(brh_b, hx_b, nc.vector)
            vboxh(brh_f2, hx_f2, nc.vector)
            vboxh(brh_b2, hx_b2, nc.vector)

            # ---- arsqrt window 1: attention rstd ----
            for c in range(NCH):
                nc.scalar.activation(rstd_sb[:, csl(c)], varp[c],
                                     AF.Abs_reciprocal_sqrt,
                                     bias=eps_sb[0:2, :])

            # u = s2 - s1^2/N ; rstd = arsqrt(u/(N-1))
            def stats1(s1, s2, v_t, eng1, eng2):
                eng1.tensor_mul(v_t, s1, s1)
                nc.vector.tensor_scalar(v_t, v_t, -1.0 / NVAR, None, ALU.mult)
                eng2.tensor_add(v_t, v_t, s2)

            stats1(brh_f, brh_f2, v_f, nc.vector, nc.vector)
            stats1(brh_b, brh_b2, v_b, nc.vector, nc.vector)
            nc.scalar.activation(r_f.rearrange("p b c t -> p (b c t)"),
                                 v_f.rearrange("p b c t -> p (b c t)"),
                                 AF.Abs_reciprocal_sqrt,
                                 scale=1.0 / (NVAR - 1))
            nc.scalar.activation(r_b.rearrange("p b c t -> p (b c t)"),
                                 v_b.rearrange("p b c t -> p (b c t)"),
                                 AF.Abs_reciprocal_sqrt,
                                 scale=1.0 / (NVAR - 1))
            nc.vector.tensor_scalar(m_f, brh_f, 1.0 / NVAR, None, ALU.mult)
            nc.vector.tensor_scalar(m_b, brh_b, 1.0 / NVAR, None, ALU.mult)
            nc.vector.tensor_mul(sd_b, v_b, r_b)
            nc.vector.tensor_scalar(sd_b, sd_b, 1.0 / (NVAR - 1),
                                    None, ALU.mult)

            # CS = cs * rstd
            for c in range(NCH):
                nc.vector.tensor_mul(CS_sb[:, csl(c)], cs_sb[:, csl(c)],
                                     rstd_sb[:, csl(c)])

            # ================= scores / exp / contraction =================
            ndp = {}
            for c in range(NCH):
                sc_ps = psS.tile([128, 512], f32, tag="sc", name=f"sc_{c}")
                nc.tensor.matmul(sc_ps[:, 0:PCS[c][1]], k2_sb,
                                 CS_sb[:, csl(c)], start=True, stop=True)
                if c >= 1:
                    ndp[c - 1] = small(c - 1, 4, "nd")
                    nc.tensor.matmul(ndp[c - 1], cj_sb,
                                     ex_sb[:, csl(c - 1)],
                                     start=True, stop=True)
                    nc.scalar.activation(nd_sb[:, csl(c - 1)], ndp[c - 1],
                                         AF.Copy)
                nc.scalar.activation(ex_sb[:, csl(c)],
                                     sc_ps[:, 0:PCS[c][1]], AF.Exp)
            ndp[NCH - 1] = small(NCH - 1, 4, "nd")
            nc.tensor.matmul(ndp[NCH - 1], cj_sb, ex_sb[:, csl(NCH - 1)],
                             start=True, stop=True)
            nc.scalar.activation(nd_sb[:, csl(NCH - 1)], ndp[NCH - 1],
                                 AF.Copy)

            # ================= mlp_out tails =================
            hd = {}
            for c in range(NCH):
                t = psA.tile([38, 512], f32, tag="sm", name=f"hd_{c}")
                hd[c] = t[:, 0:PCS[c][1]]
                nc.tensor.matmul(hd[c], hdsel_sb, nd_sb[:, csl(c)],
                                 start=True, stop=True)
                nc.vector.reciprocal(r6_sb[:, csl(c)].bitcast(f32),
                                     hd[c][32:38, :])
                nc.vector.tensor_mul(xg_sb[:, csl(c)], hd[c][0:6, :],
                                     r6_sb[:, csl(c)])
            for c in range(NCH):
                nc.scalar.activation(h2_sb[:, csl(c)], xg_sb[0:6, csl(c)],
                                     AF.Gelu_apprx_tanh, bias=mob1_sb)
            v2p = {}
            for c in range(NCH):
                y2ps = small(c, 3, "y2")
                nc.tensor.matmul(y2ps, mo2cT_sb, h2_sb[:, csl(c)],
                                 start=True, stop=True)
                nc.vector.tensor_scalar(y2_sb[0:3, csl(c)], y2ps,
                                        mob2c_sb, None, ALU.add)
                nc.gpsimd.tensor_mul(sq2_sb[:, csl(c)], y2_sb[0:3, csl(c)],
                                     y2_sb[0:3, csl(c)])
                v2p[c] = small(c, 3, "v2")
                nc.tensor.matmul(v2p[c], ones33_sb, sq2_sb[:, csl(c)],
                                 start=True, stop=True)

            # ---- arsqrt window 2: ln2 rstd ----
            for c in range(NCH):
                nc.scalar.activation(r32_sb[:, csl(c)], v2p[c],
                                     AF.Abs_reciprocal_sqrt,
                                     bias=eps_sb[0:3, :])
            for c in range(NCH):
                op_ps = small(c, 3, "op")
                nc.tensor.matmul(op_ps, fuse4_sb, y2_sb[:, csl(c)],
                                 start=True, stop=True)
                nc.vector.tensor_mul(oa_sb[:, csl(c)], op_ps,
                                     r32_sb[:, csl(c)])
            nc.sync.dma_start(out=d_oa[:, :], in_=oa_sb.bitcast(f32))

            # ---- PSF mvn/adain/convs (w-layout) ----
            ctr = slice(PAD, PAD + HSL)
            nc.vector.tensor_sub(xnf, pf[:, :, :, ctr], m_f)
            nc.vector.tensor_mul(xnf, xnf, r_f)
            nc.vector.tensor_sub(xnb, pb[:, :, :, ctr], m_b)
            nc.vector.tensor_mul(xnb, xnb, r_b)
            nc.vector.tensor_mul(xad, xnf, sd_b)
            nc.vector.tensor_add(xad, xad, m_b)

            def conv3(dst, src, wbase, bbase, name):
                for co in range(3):
                    dco = dst[:, :, co, :]
                    t = psft.tile([W, B, HSL], f32, tag="conv_t",
                                  name=f"cv_{name}_{co}")
                    t2 = psft.tile([W, B, HSL], f32, tag="conv_t2",
                                   name=f"cv2_{name}_{co}")
                    w0 = cw_sb[:, wbase + co * 3:wbase + co * 3 + 1]
                    w1 = cw_sb[:, wbase + co * 3 + 1:wbase + co * 3 + 2]
                    w2 = cw_sb[:, wbase + co * 3 + 2:wbase + co * 3 + 3]
                    if bbase is not None:
                        nc.vector.tensor_scalar(
                            dco, src[:, :, 0, :], w0,
                            cw_sb[:, bbase + co:bbase + co + 1],
                            ALU.mult, ALU.add)
                    else:
                        nc.vector.tensor_scalar(dco, src[:, :, 0, :],
                                                w0, None, ALU.mult)
                    nc.vector.tensor_scalar(t, src[:, :, 1, :], w1,
                                            None, ALU.mult)
                    nc.vector.tensor_scalar(t2, src[:, :, 2, :], w2,
                                            None, ALU.mult)
                    nc.vector.tensor_add(dco, dco, t)
                    nc.vector.tensor_add(dco, dco, t2)

            conv3(EE, xad, 0, 45, "EE")
            conv3(FF, xnf, 9, 48, "FF")
            conv3(GG, xnb, 18, 51, "GG")
            conv3(HH, pb[:, :, :, ctr], 27, 54, "HH")

            # cosine-sim chain (muls DVE, adds Pool)
            tmc = psft.tile([W, B, C, HSL], f32, tag="tmc", name="tmc1")
            nc.vector.tensor_mul(tmc, FF, GG)
            nc.vector.tensor_add(dot, tmc[:, :, 0, :], tmc[:, :, 1, :])
            nc.vector.tensor_add(dot, dot, tmc[:, :, 2, :])
            tmf = psft.tile([W, B, C, HSL], f32, tag="tmc", name="tmc2")
            nc.vector.tensor_mul(tmf, FF, FF)
            nc.vector.tensor_add(f2, tmf[:, :, 0, :], tmf[:, :, 1, :])
            nc.vector.tensor_add(f2, f2, tmf[:, :, 2, :])
            tmg = psft.tile([W, B, C, HSL], f32, tag="tmc", name="tmc3")
            nc.vector.tensor_mul(tmg, GG, GG)
            nc.vector.tensor_add(g2, tmg[:, :, 0, :], tmg[:, :, 1, :])
            nc.vector.tensor_add(g2, g2, tmg[:, :, 2, :])
            nc.vector.tensor_mul(f2, f2, g2)          # F2*G2

            # cosine rsqrt via the u32 bit trick + 1 Newton step (DVE)
            trq = psft.tile([W, B, HSL], f32, tag="trq", name="trq")
            g2u = g2.rearrange("p b t -> p (b t)").bitcast(u32)
            f2u = f2.rearrange("p b t -> p (b t)").bitcast(u32)
            nc.vector.tensor_scalar(g2u, f2u, 1, None,
                                    ALU.logical_shift_right)
            nc.vector.tensor_sub(g2u, magic_sb[:, 0:B * HSL], g2u)
            nc.vector.tensor_mul(trq, g2, g2)
            nc.vector.tensor_mul(trq, trq, f2)
            tqf = trq.rearrange("p b t -> p (b t)")
            nc.vector.tensor_scalar(tqf, tqf, -0.5, 1.5, ALU.mult, ALU.add)
            nc.vector.tensor_mul(g2, g2, trq)
            nc.vector.tensor_mul(S, dot, g2)

            # ---- minmax partials + fuse pieces + out ----
            rmx = psft.tile([W, B], f32, tag="rmx", name="rmx")
            rmn = psft.tile([W, B], f32, tag="rmn", name="rmn")
            nS = psft.tile([W, B, HSL], f32, tag="nS", name="nS")
            nc.vector.tensor_scalar(nS, S, -1.0, None, ALU.mult)
            nc.vector.tensor_reduce(rmx, S, axis=AX.X, op=ALU.max)
            nc.vector.tensor_reduce(rmn, nS, axis=AX.X, op=ALU.max)
            mmt = psft.tile([W, 2 * B], f32, tag="mmt", name="mmt")
            nc.gpsimd.partition_all_reduce(mmt[:, 0:B], rmx, W, ROP.max)
            nc.gpsimd.partition_all_reduce(mmt[:, B:2 * B], rmn, W, ROP.max)
            nc.sync.dma_start(out=d_mm[0:1, :], in_=mmt[0:1, :])

            nc.vector.tensor_sub(A3, EE, HH)
            for cc in range(3):
                nc.vector.tensor_mul(P3[:, :, cc, :], A3[:, :, cc, :], S)
            conv3(psO[:, 0], P3, 36, None, "cP")
            conv3(psO[:, 1], A3, 36, None, "cA")
            conv3(psO[:, 2], HH, 36, None, "cH")
            nc.sync.dma_start(
                out=d_ps[:, :],
                in_=psO.rearrange("p g b c t -> p (g b c t)"))
    nc.compile()
    return nc


_CACHED = {}


def _fit_supercodes(inputs):
    """FFT deconvolution of the codebook softmax onto M von-Mises kernels.
    Pure weight transform (depends only on weights, not pixel data)."""
    f64 = lambda k: np.asarray(inputs[k], np.float64)
    bg = f64("bg_embed")
    q_w, k_w, v_w = f64("q_w"), f64("k_w"), f64("v_w")
    n1_g, n1_b = f64("n1_g"), f64("n1_b")
    kT = (k_w @ bg) * SCALE
    v = bg.T @ v_w.T
    v_ext = np.concatenate([v, np.ones((NE, 1))], 1)
    u = np.array([1.0, -1.0, 0.0]) / np.sqrt(2)
    w = np.array([1.0, 1.0, -2.0]) / np.sqrt(6)
    alpha = (np.sqrt(3) * (n1_g * u) @ q_w.T) @ kT
    beta = (np.sqrt(3) * (n1_g * w) @ q_w.T) @ kT
    gamma = (n1_b @ q_w.T) @ kT

    G = 4096
    th = 2 * np.pi * np.arange(G) / G
    Ct, St = np.cos(th), np.sin(th)
    T = np.zeros((G, 4))
    for s in range(0, G, 512):
        sl = slice(s, s + 512)
        T[sl] = np.exp(np.outer(Ct[sl], alpha) + np.outer(St[sl], beta)
                       + gamma) @ v_ext
    Th = np.fft.rfft(T, axis=0) / G
    Kg = np.exp(RTIL * (np.cos(th) - 1.0))
    Kh = np.fft.rfft(Kg) / G
    ch = np.zeros((G // 2 + 1, 4), complex)
    ch[:M_CUT + 1] = Th[:M_CUT + 1] / Kh[:M_CUT + 1, None]
    chM = np.zeros((M_SC // 2 + 1, 4), complex)
    chM[:M_CUT + 1] = ch[:M_CUT + 1]
    cj = np.fft.irfft(chM, n=M_SC, axis=0) * M_SC
    phi = 2 * np.pi * np.arange(M_SC) / M_SC
    # absolute-scale fix on a few grid points (ratio-invariant)
    km = np.exp(RTIL * (np.cos(th[:64, None] - phi[None, :]) - 1.0))
    cj /= np.median((km @ cj[:, 3]) / T[:64, 3])
    # device scores are rtil*(C cos + S sin) WITHOUT the -rtil: fold e^-rtil
    cj *= np.exp(-RTIL)
    k2 = np.stack([RTIL * np.cos(phi), RTIL * np.sin(phi)])  # [2, M]
    # (u,w)/sqrt(3) projection of the centered mlp_in output
    U2 = np.stack([u, w]) / np.sqrt(3)
    return (np.ascontiguousarray(k2, np.float32),
            np.ascontiguousarray(cj, np.float32), U2)


def _prepare_in_maps(inputs):
    f = lambda k: np.asarray(inputs[k], np.float32)
    front, back = f("front"), f("back")
    mi_w1, mi_b1 = f("mi_w1"), f("mi_b1")
    mi_w2, mi_b2 = f("mi_w2"), f("mi_b2")
    mo_w1, mo_b1 = f("mo_w1"), f("mo_b1")
    mo_w2, mo_b2 = f("mo_w2"), f("mo_b2")
    n2_g, n2_b = f("n2_g"), f("n2_b")
    e_w, e_b = f("e_w"), f("e_b")
    f_w, f_b = f("f_w"), f("f_b")
    g_w, g_b = f("g_w"), f("g_b")
    h_w, h_b = f("h_w"), f("h_b")
    fuse_w, fuse_b = f("fuse_w"), f("fuse_b")

    k2, cj, U2 = _fit_supercodes(inputs)

    w2c = mi_w2 - mi_w2.mean(0, keepdims=True)
    b2c = mi_b2 - mi_b2.mean()
    csw = (U2 @ w2c.astype(np.float64)).astype(np.float32)      # [2, 6]
    csb = (U2 @ b2c.astype(np.float64)).astype(np.float32)      # [2]
    mo2c = mo_w2 - mo_w2.mean(0, keepdims=True)
    mob2c = mo_b2 - mo_b2.mean()

    hh, ww = np.meshgrid(np.arange(W), np.arange(W), indexing="ij")
    band = (np.abs(hh - ww) <= PAD).astype(np.float32)
    cw = np.concatenate([e_w.ravel(), f_w.ravel(), g_w.ravel(),
                         h_w.ravel(), fuse_w[:, 3:6].ravel(),
                         e_b, f_b, g_b, h_b])

    wpack = np.zeros((8, 64), np.float32)
    wpack[0:3, 0:6] = mi_w1.T
    wpack[0:6, 6:8] = csw.T
    wpack[6, 6:8] = csb
    wpack[0:2, 8:10] = 1.0
    wpack[0:3, 10:13] = fuse_w[:, 0:3].T * n2_g[:, None]
    wpack[3, 10:13] = fuse_b + fuse_w[:, 0:3] @ n2_b
    wpack[0:3, 13:16] = 1.0 / 3.0
    wpack[0:6, 16:19] = mo2c.T
    wpack[0:3, 19:25] = mo_w1.T
    wpack[3, 51:57] = 1.0
    vpack = np.zeros((6, 6), np.float32)
    vpack[0:6, 0] = mi_b1
    vpack[0:6, 2] = mo_b1
    vpack[0:3, 3] = mob2c
    vpack[:, 5] = 1e-5

    common = dict(
        k2=k2, cj=cj, band=band,
        wpack=wpack, vpack=vpack,
        cw=np.ascontiguousarray(cw, np.float32),
    )
    common = {k: np.ascontiguousarray(v2, np.float32)
              for k, v2 in common.items()}

    frp = np.pad(front, ((0, 0), (0, 0), (PAD, PAD), (0, 0)))
    bkp = np.pad(back, ((0, 0), (0, 0), (PAD, PAD), (0, 0)))
    in_maps = []
    for i in range(N_CORES):
        sl = front[:, :, HSL * i:HSL * (i + 1), :]          # [B,3,12,96]
        xcm = np.ascontiguousarray(np.concatenate(
            [sl.transpose(1, 0, 2, 3).reshape(D, PIX),
             np.ones((1, PIX), np.float32)]), np.float32)
        fps = np.ascontiguousarray(
            frp[:, :, HSL * i:HSL * i + HT, :].transpose(3, 0, 1, 2)
            .reshape(W, BCT), np.float32)
        bps = np.ascontiguousarray(
            bkp[:, :, HSL * i:HSL * i + HT, :].transpose(3, 0, 1, 2)
            .reshape(W, BCT), np.float32)
        in_maps.append(dict(common, front_cm=xcm, fpsf=fps, bpsf=bps))
    return in_maps


def _gather_output(res):
    # global min/max of the cosine map S from the per-core partials
    mms = np.stack([np.asarray(res.results[i]["mm_out"], np.float32)
                    for i in range(N_CORES)])            # [8, 1, 2B]
    mx = mms[:, 0, 0:B].max(0)                           # [B]
    mn = -mms[:, 0, B:2 * B].max(0)                      # [B]
    ri = 1.0 / (mx - mn)
    out = np.zeros((B, C, H, W), np.float32)
    for i in range(N_CORES):
        ps = np.asarray(res.results[i]["psf_out"], np.float32)
        ps = ps.reshape(W, 3, B, 3, HSL).transpose(1, 2, 3, 4, 0)
        cP, cA, cH = ps[0], ps[1], ps[2]                 # [B, 3, 12, 96]
        w_mn = mn[:, None, None, None]
        w_ri = ri[:, None, None, None]
        ff = (cP - w_mn * cA) * w_ri + cH
        oa = np.asarray(res.results[i]["out_a"],
                        np.float32).reshape(D, B, HSL, W)
        out[:, :, HSL * i:HSL * (i + 1), :] = ff + oa.transpose(1, 0, 2, 3)
    return out


def kernel(**inputs):
    import sys
    if "/opt/trn_rl_repo" not in sys.path:
        sys.path.insert(0, "/opt/trn_rl_repo")
    from concourse.bass_utils import run_bass_kernel_spmd

    in_maps = _prepare_in_maps(inputs)
    if "nc" not in _CACHED:
        _CACHED["nc"] = _build_program()
    nc = _CACHED["nc"]

    res = run_bass_kernel_spmd(nc, in_maps, core_ids=list(range(N_CORES)))
    return _gather_output(res)
